# revision 4
# baseline (speedup 1.0000x reference)
"""Cross-attention kernel for Trainium2 (Bass/Tile), 8-core data-parallel.

Reference computation (per batch element b):
    q = x @ Wq.T ; k = ctx @ Wk.T ; v = ctx @ Wv.T
    out = softmax((q @ k.T) * D**-0.5) @ v

Shapes: x [8, 2048, 1024], context [8, 2048, 1024], Wq/Wk/Wv [1024, 1024].

Strategy: pure data-parallel -- one batch element per NeuronCore, no
collectives. All matmuls in bf16 with fp32 PSUM accumulation.

Since softmax((q k^T) * s) only needs q k^T = x (Wq^T Wk) ctx^T, we never
materialize q or k: W' = Wq^T Wk is computed from the *natural* weight
layouts (contraction over the out-feature axis, which is already on
partitions), then yT = W'^T x^T and dots = yT^T ctx^T. This kills the k
projection and all Wq/Wk transposes.

PE-offload changes vs the 452us baseline:
  * All activation/weight transposes (ctx^T, x^T, Wv^T) run on the DMA
    engines via the XBAR transpose instruction (dma_start_transpose,
    bf16 SBUF->SBUF) instead of PE transpose-mode matmuls + DVE drains.
    Transposed-block group tiles are laid out [128, ntiles, 4, 128] so
    every transpose destination is a contiguous 512-element span (the
    XBAR path requires contiguous destinations).
  * Softmax denominators come from an fp32 running sum of the exp tiles
    on the DVE (15 chained adds per half) + one tiny fp32 ones-matmul
    per 128-row s-tile, replacing 64 N=512 ones-matmuls and the PE
    transposes of the row-sum vectors.
  * ~18 dummy matmuls on a zero tile warm the PE's HAM clock gate during
    the initial DMA wait so real matmuls start at 2.4 GHz.
Softmax runs without max-subtraction (logits are O(5) for unit-normal
inputs); exp comes straight out of PSUM on the Scalar engine with the
1/32 scale folded in, and row normalization is applied after the attn@v
matmul since that matmul is linear in attn.
"""

from contextlib import ExitStack

import numpy as np

B = 8
S = 2048  # query length
T = 2048  # key/value length
D = 1024  # model dim
P = 128
SCALE = float(D) ** -0.5

N_ST = S // P  # 16 query tiles
N_TT = T // P  # 16 key tiles
N_DT = D // P  # 8 contraction chunks
NPROJ = D // 512  # 2 x 512-wide chunks for [., 1024] outputs
NSB = 4  # x is processed in 4 s-blocks of 512 rows for the yT projection


def _emit_body(tc, x, ctxt, wq, wk, wv, out):
    import concourse.mybir as mybir

    fp32 = mybir.dt.float32
    bf16 = mybir.dt.bfloat16
    nc = tc.nc

    with ExitStack() as ctx:
        const = ctx.enter_context(tc.tile_pool(name="const", bufs=1))
        stage = ctx.enter_context(tc.tile_pool(name="stage", bufs=3))
        castp = ctx.enter_context(tc.tile_pool(name="castp", bufs=4))
        wvtp = ctx.enter_context(tc.tile_pool(name="wvtp", bufs=2))
        ctxp = ctx.enter_context(tc.tile_pool(name="ctxp", bufs=2))
        vp = ctx.enter_context(tc.tile_pool(name="vp", bufs=16))
        wnp = ctx.enter_context(tc.tile_pool(name="wnp", bufs=16))
        wpp = ctx.enter_context(tc.tile_pool(name="wpp", bufs=8))
        xop = ctx.enter_context(tc.tile_pool(name="xop", bufs=4))
        ytp = ctx.enter_context(tc.tile_pool(name="ytp", bufs=8))
        spp = ctx.enter_context(tc.tile_pool(name="spp", bufs=2))
        smp = ctx.enter_context(tc.tile_pool(name="smp", bufs=8))

        ones_f = const.tile([P, 1], fp32, name="ones_f")
        nc.vector.memset(ones_f, 1.0)
        junk = const.tile([P, 512], bf16, name="junk")
        nc.vector.memset(junk, 0.0)

        def load_cast(dram_rows, nm, eng="v", pool=None, tag="cast"):
            """DMA one fp32 [128, D] row-tile and cast it to bf16."""
            st_t = stage.tile([P, D], fp32, name=f"ld_{nm}", tag="stage")
            nc.sync.dma_start(out=st_t, in_=dram_rows)
            bt = (pool or castp).tile([P, D], bf16, name=f"bf_{nm}", tag=tag)
            if eng == "v":
                nc.vector.tensor_copy(out=bt, in_=st_t)
            else:
                nc.scalar.copy(out=bt, in_=st_t)
            return bt

        def emit_T(bt, grp, rt):
            """XBAR-transpose the 8 128x128 blocks of a bf16 [128, D] tile
            into the two group tiles: grp[g][:, rt, j, c] = bt[c, (4g+j)*128+p].
            Destination spans are contiguous 512-element runs."""
            for g in range(2):
                nc.sync.dma_start_transpose(
                    out=grp[g][:, rt, :, :], in_=bt[:, g * 512 : (g + 1) * 512]
                )

        # Group tiles: [p, tile, j, c] so each transpose dst is contiguous.
        wvg = [
            wvtp.tile([P, N_DT, 4, P], bf16, name=f"wvg{g}", tag="wvg")
            for g in range(2)
        ]
        ctxg = [
            ctxp.tile([P, N_TT, 4, P], bf16, name=f"ctxg{g}", tag="ctxg")
            for g in range(2)
        ]
        v = [vp.tile([P, D], bf16, name=f"v{t}", tag="v") for t in range(N_TT)]

        def prep_wv(rt):
            wb = load_cast(wv[rt * P : (rt + 1) * P, :], f"wv{rt}")
            emit_T(wb, wvg, rt)

        def prep_ctx(rt):
            cb = load_cast(ctxt[rt * P : (rt + 1) * P, :], f"c{rt}")
            emit_T(cb, ctxg, rt)

        with tc.tile_pool(name="psumA", bufs=1, space="PSUM") as psA:
            # PE warmup: ~18 junk matmuls (~3.8us) release the HAM clock
            # gate while the first input tiles stream in.
            wu = psA.tile([P, 512], fp32, name="wu", tag="wu", bufs=1)
            for i in range(18):
                nc.tensor.matmul(wu, junk[:, :P], junk, start=True, stop=True)

            # ---- Wv^T + ctx^T prep, v = ctx @ Wv^T chases the transposes --
            # Wv rows 0-511 first so v[tt][ne=0] can start after just 4 Wv
            # tiles + one ctx tile; remaining Wv tiles stream in behind.
            for rt in range(4):
                prep_wv(rt)
            prep_ctx(0)
            for rt in range(4, N_DT):
                prep_wv(rt)
            prep_ctx(1)

            wqn = []
            wkn = []
            for rt in range(N_TT):
                # keep ctx prep two tiles ahead of the consuming matmuls
                if rt + 2 < N_TT:
                    prep_ctx(rt + 2)
                # prefetch Wq/Wk behind the ctx stream (needed at ~60us)
                if 9 <= rt < 13:
                    e = (rt - 9) * 2
                    wqn.append(load_cast(wq[e * P : (e + 1) * P, :], f"wq{e}", "s", wnp, "wn"))
                    wqn.append(
                        load_cast(
                            wq[(e + 1) * P : (e + 2) * P, :], f"wq{e + 1}", "s",
                            wnp, "wn",
                        )
                    )
                elif rt >= 13:
                    e = (rt - 13) * 2
                    wkn.append(load_cast(wk[e * P : (e + 1) * P, :], f"wk{e}", "s", wnp, "wn"))
                    wkn.append(
                        load_cast(
                            wk[(e + 1) * P : (e + 2) * P, :], f"wk{e + 1}", "s",
                            wnp, "wn",
                        )
                    )
                tt = rt  # v = ctx @ Wv^T, natural layout [t, e]
                for ne in range(NPROJ):
                    ps = psA.tile(
                        [P, 512], fp32, name=f"pv{tt}_{ne}", tag="proj", bufs=4
                    )
                    for d in range(N_DT):
                        nc.tensor.matmul(
                            ps,
                            ctxg[d // 4][:, tt, d % 4, :],
                            wvg[d // 4][:, 4 * ne : 4 * ne + 4, d % 4, :],
                            start=(d == 0),
                            stop=(d == N_DT - 1),
                        )
                    nc.scalar.copy(out=v[tt][:, ne * 512 : (ne + 1) * 512], in_=ps)
            wkn.append(load_cast(wk[6 * P : 7 * P, :], "wk6", "s", wnp, "wn"))
            wkn.append(load_cast(wk[7 * P : 8 * P, :], "wk7", "s", wnp, "wn"))

            # ---- W' = Wq^T @ Wk from natural layouts ----
            wpb = [
                wpp.tile([P, D], bf16, name=f"wp{i}", tag="wp") for i in range(N_DT)
            ]
            for it in range(N_DT):
                for jn in range(NPROJ):
                    ps = psA.tile(
                        [P, 512], fp32, name=f"pw{it}_{jn}", tag="proj", bufs=4
                    )
                    for e in range(N_DT):
                        nc.tensor.matmul(
                            ps,
                            wqn[e][:, it * P : (it + 1) * P],
                            wkn[e][:, jn * 512 : (jn + 1) * 512],
                            start=(e == 0),
                            stop=(e == N_DT - 1),
                        )
                    nc.scalar.copy(out=wpb[it][:, jn * 512 : (jn + 1) * 512], in_=ps)

            # ---- yT = (x @ W')^T = W'^T x^T, streamed over 4 s-blocks ----
            yt = [
                ytp.tile([P, S], bf16, name=f"yt{j}", tag="yt") for j in range(N_DT)
            ]
            xtb = {}

            def prep_x(sb):
                xtb[sb] = [
                    xop.tile([P, 4, 4, P], bf16, name=f"xtb{sb}_{g}", tag="xtb")
                    for g in range(2)
                ]
                for r in range(4):
                    rt = 4 * sb + r
                    xb = load_cast(x[rt * P : (rt + 1) * P, :], f"x{rt}")
                    for g in range(2):
                        nc.sync.dma_start_transpose(
                            out=xtb[sb][g][:, r, :, :],
                            in_=xb[:, g * 512 : (g + 1) * 512],
                        )

            prep_x(0)
            for sb in range(NSB):
                if sb + 1 < NSB:
                    prep_x(sb + 1)
                for jt in range(N_DT):
                    ps = psA.tile(
                        [P, 512], fp32, name=f"py{sb}_{jt}", tag="proj", bufs=4
                    )
                    for i in range(N_DT):
                        nc.tensor.matmul(
                            ps,
                            wpb[i][:, jt * P : (jt + 1) * P],
                            xtb[sb][i // 4][:, :, i % 4, :],
                            start=(i == 0),
                            stop=(i == N_DT - 1),
                        )
                    nc.scalar.copy(out=yt[jt][:, sb * 512 : (sb + 1) * 512], in_=ps)
                del xtb[sb]

        # ---- attention ----
        # dots is produced TRANSPOSED: dotsT[t_tile, s] = sum_d ctxT[d, t] *
        # yT[d, s] (same operands as dots, roles swapped), so exp output IS
        # attn^T and the attn@v matmul needs no transposes at all. Softmax
        # denominators: a running fp32 elementwise sum of the 16 attn^T
        # tiles on the DVE, then one tiny fp32 ones-matmul per s-tile flips
        # the 128-partition reduction into a [128, 1] column. S is processed
        # in 2 halves of 1024 so attn^T fits the 16 slots Wq/Wk vacated.
        SH = S // 2
        with tc.tile_pool(name="psumB", bufs=1, space="PSUM") as psB:
            for h in range(2):
                atT = []
                spart = spp.tile([P, SH], fp32, name=f"sp{h}", tag="sp")
                for tt in range(N_TT):
                    at = wnp.tile([P, SH], bf16, name=f"atT{h}_{tt}", tag="wn")
                    for ns in range(SH // 512):
                        ps = psB.tile(
                            [P, 512], fp32, name=f"pd{h}_{tt}_{ns}", tag="dots",
                            bufs=4,
                        )
                        for d in range(N_DT):
                            nc.tensor.matmul(
                                ps,
                                ctxg[d // 4][:, tt, d % 4, :],
                                yt[d][:, h * SH + ns * 512 : h * SH + (ns + 1) * 512],
                                start=(d == 0),
                                stop=(d == N_DT - 1),
                            )
                        nc.scalar.activation(
                            out=at[:, ns * 512 : (ns + 1) * 512],
                            in_=ps,
                            func=mybir.ActivationFunctionType.Exp,
                            scale=SCALE,
                        )
                    atT.append(at)
                    # fp32 running column-sum on the DVE chases the exps
                    if tt == 0:
                        nc.vector.tensor_copy(out=spart, in_=at)
                    else:
                        nc.vector.tensor_add(out=spart, in0=spart, in1=at)

                recips = []
                for sl in range(N_ST // 2):
                    psc = psB.tile(
                        [P, 1], fp32, name=f"psc{h}_{sl}", tag="scol", bufs=1
                    )
                    nc.tensor.matmul(
                        psc,
                        spart[:, sl * P : (sl + 1) * P],
                        ones_f,
                        start=True,
                        stop=True,
                    )
                    recip = smp.tile([P, 1], fp32, name=f"rc{h}_{sl}", tag="recip")
                    nc.vector.reciprocal(out=recip, in_=psc)
                    recips.append(recip)

                for sl in range(N_ST // 2):
                    st = h * (N_ST // 2) + sl
                    out_sb = xop.tile([P, D], fp32, name=f"o{st}", tag="xtb")
                    for ne in range(NPROJ):
                        ps = psB.tile(
                            [P, 512], fp32, name=f"pav{st}_{ne}", tag="av", bufs=3
                        )
                        for tt in range(N_TT):
                            nc.tensor.matmul(
                                ps,
                                atT[tt][:, sl * P : (sl + 1) * P],
                                v[tt][:, ne * 512 : (ne + 1) * 512],
                                start=(tt == 0),
                                stop=(tt == N_TT - 1),
                            )
                        nc.scalar.mul(
                            out=out_sb[:, ne * 512 : (ne + 1) * 512],
                            in_=ps,
                            mul=recips[sl],
                        )
                    nc.sync.dma_start(
                        out=out[st * P : (st + 1) * P, :], in_=out_sb
                    )


def build_nc():
    import concourse.mybir as mybir
    import concourse.tile as tile
    from concourse import bacc

    fp32 = mybir.dt.float32
    nc = bacc.Bacc("TRN2", target_bir_lowering=False, debug=False)
    x = nc.dram_tensor("x", [S, D], fp32, kind="ExternalInput").ap()
    ctxt = nc.dram_tensor("context", [T, D], fp32, kind="ExternalInput").ap()
    wq = nc.dram_tensor("Wq", [D, D], fp32, kind="ExternalInput").ap()
    wk = nc.dram_tensor("Wk", [D, D], fp32, kind="ExternalInput").ap()
    wv = nc.dram_tensor("Wv", [D, D], fp32, kind="ExternalInput").ap()
    out = nc.dram_tensor("out", [S, D], fp32, kind="ExternalOutput").ap()
    with tile.TileContext(nc) as tc:
        _emit_body(tc, x, ctxt, wq, wk, wv, out)
    nc.compile()
    return nc


_CACHED_NC = None


def kernel(**inputs):
    global _CACHED_NC
    from concourse.bass_utils import run_bass_kernel_spmd

    x = np.ascontiguousarray(np.asarray(inputs["x"], dtype=np.float32))
    ctxt = np.ascontiguousarray(np.asarray(inputs["context"], dtype=np.float32))
    wq = np.ascontiguousarray(np.asarray(inputs["Wq"], dtype=np.float32))
    wk = np.ascontiguousarray(np.asarray(inputs["Wk"], dtype=np.float32))
    wv = np.ascontiguousarray(np.asarray(inputs["Wv"], dtype=np.float32))

    if _CACHED_NC is None:
        _CACHED_NC = build_nc()
    nc = _CACHED_NC

    in_maps = [
        {"x": x[b], "context": ctxt[b], "Wq": wq, "Wk": wk, "Wv": wv}
        for b in range(B)
    ]
    res = run_bass_kernel_spmd(nc, in_maps, core_ids=list(range(B)))
    return np.stack([res.results[b]["out"] for b in range(B)], axis=0)


# revision 19
# speedup vs baseline: 1.3744x; 1.3744x over previous
"""Cross-attention kernel for Trainium2 (Bass/Tile), 8-core data-parallel.

Reference computation (per batch element b):
    q = x @ Wq.T ; k = ctx @ Wk.T ; v = ctx @ Wv.T
    out = softmax((q @ k.T) * D**-0.5) @ v

Shapes: x [8, 2048, 1024], context [8, 2048, 1024], Wq/Wk/Wv [1024, 1024].

Strategy: pure data-parallel -- one batch element per NeuronCore, no
collectives. All matmuls in bf16 with fp32 PSUM accumulation.

Since softmax((q k^T) * s) only needs q k^T = x (Wq^T Wk) ctx^T, we never
materialize q or k: W' = Wq^T Wk is computed from the *natural* weight
layouts (contraction over the out-feature axis, which is already on
partitions), then yT = W'^T x^T and dots = yT^T ctx^T. This kills the k
projection and all Wq/Wk transposes.

Changes vs the 452us baseline:
  * Softmax denominators come from an fp32 running sum of the exp tiles
    on the DVE (15 chained adds per half) + one tiny fp32 ones-matmul
    per 128-row s-tile, replacing 64 N=512 ones-matmuls and the PE
    transposes of the row-sum vectors. (XBAR DMA transposes were tried
    for the ctx/x/Wv transposes and are a large regression: 512 tiny
    descriptors + ~1.3us issue cost per [128,512] transpose starve the
    load pipeline. PE transpose-mode is the right engine for these.)
  * Transposed-block group tiles are [128, ntiles, 4, 128] so each
    transpose group drains to a contiguous span and matmul stationary
    slices are contiguous 128-element runs.
  * ~18 dummy matmuls on a zero tile warm the PE's HAM clock gate during
    the initial DMA wait so real matmuls start at 2.4 GHz.
  * Wq/Wk loads are interleaved behind the ctx stream on the DMA queue
    so W' can start the moment the v projection retires.
Softmax runs without max-subtraction (logits are O(5) for unit-normal
inputs); exp comes straight out of PSUM on the Scalar engine with the
1/32 scale folded in, and row normalization is applied after the attn@v
matmul since that matmul is linear in attn.
"""

from contextlib import ExitStack

import numpy as np

B = 8
S = 2048  # query length
T = 2048  # key/value length
D = 1024  # model dim
P = 128
SCALE = float(D) ** -0.5

N_ST = S // P  # 16 query tiles
N_TT = T // P  # 16 key tiles
N_DT = D // P  # 8 contraction chunks
NPROJ = D // 512  # 2 x 512-wide chunks for [., 1024] outputs
NSB = 4  # x is processed in 4 s-blocks of 512 rows for the yT projection


def _emit_body(tc, x, ctxt, wq, wk, wv, out):
    import concourse.mybir as mybir
    from concourse.masks import make_identity

    fp32 = mybir.dt.float32
    bf16 = mybir.dt.bfloat16
    nc = tc.nc

    with ExitStack() as ctx:
        const = ctx.enter_context(tc.tile_pool(name="const", bufs=1))
        stage = ctx.enter_context(tc.tile_pool(name="stage", bufs=3))
        castp = ctx.enter_context(tc.tile_pool(name="castp", bufs=6))
        wvtp = ctx.enter_context(tc.tile_pool(name="wvtp", bufs=2))
        ctxp = ctx.enter_context(tc.tile_pool(name="ctxp", bufs=2))
        vp = ctx.enter_context(tc.tile_pool(name="vp", bufs=16))
        wnp = ctx.enter_context(tc.tile_pool(name="wnp", bufs=16))
        wpp = ctx.enter_context(tc.tile_pool(name="wpp", bufs=8))
        xop = ctx.enter_context(tc.tile_pool(name="xop", bufs=4))
        ytp = ctx.enter_context(tc.tile_pool(name="ytp", bufs=8))
        spp = ctx.enter_context(tc.tile_pool(name="spp", bufs=1))
        smp = ctx.enter_context(tc.tile_pool(name="smp", bufs=8))

        ones_b = const.tile([P, 1], bf16, name="ones_b")
        nc.vector.memset(ones_b, 1.0)
        junk = const.tile([P, 512], bf16, name="junk")
        nc.vector.memset(junk, 0.0)
        ident_b = const.tile([P, P], bf16, name="ident_b")
        make_identity(nc, ident_b)

        def load_cast(dram_rows, nm, eng="v", pool=None, tag="cast"):
            """DMA one fp32 [128, D] row-tile and cast it to bf16."""
            st_t = stage.tile([P, D], fp32, name=f"ld_{nm}", tag="stage")
            nc.sync.dma_start(out=st_t, in_=dram_rows)
            bt = (pool or castp).tile([P, D], bf16, name=f"bf_{nm}", tag=tag)
            if eng == "v":
                nc.vector.tensor_copy(out=bt, in_=st_t)
            else:
                nc.scalar.copy(out=bt, in_=st_t)
            return bt

        def emit_T(bt, grp, rt, psum_pool, nm):
            """PE-transpose the 8 128x128 blocks of a bf16 [128, D] tile in
            2 groups of 4 sharing one PSUM bank; one copy per group drains
            into grp[g][:, rt, j, c] = bt[c, (4g+j)*128+p] (contiguous)."""
            for g in range(2):
                ps = psum_pool.tile(
                    [P, 4 * P], bf16, name=f"tp_{nm}_{g}", tag="pt", bufs=3
                )
                for j in range(4):
                    nc.tensor.transpose(
                        ps[:, j * P : (j + 1) * P],
                        bt[:, (4 * g + j) * P : (4 * g + j + 1) * P],
                        ident_b,
                    )
                nc.vector.tensor_copy(
                    out=grp[g][:, rt, :, :],
                    in_=ps.rearrange("p (j c) -> p j c", j=4),
                )

        # Group tiles: [p, tile, j, c] so each transpose dst is contiguous.
        wvg = [
            wvtp.tile([P, N_DT, 4, P], bf16, name=f"wvg{g}", tag="wvg")
            for g in range(2)
        ]
        ctxg = [
            ctxp.tile([P, N_TT, 4, P], bf16, name=f"ctxg{g}", tag="ctxg")
            for g in range(2)
        ]
        v = [vp.tile([P, D], bf16, name=f"v{t}", tag="v") for t in range(N_TT)]

        with tc.tile_pool(name="psumA", bufs=1, space="PSUM") as psA:

            def prep_wv(rt):
                wb = load_cast(wv[rt * P : (rt + 1) * P, :], f"wv{rt}")
                emit_T(wb, wvg, rt, psA, f"wv{rt}")

            def prep_ctx(rt):
                cb = load_cast(ctxt[rt * P : (rt + 1) * P, :], f"c{rt}")
                emit_T(cb, ctxg, rt, psA, f"c{rt}")

            # PE warmup: ~18 junk matmuls (~3.8us) release the HAM clock
            # gate while the first input tiles stream in.
            wu = psA.tile([P, 512], fp32, name="wu", tag="wu", bufs=1)
            for i in range(16):
                nc.tensor.matmul(wu, junk[:, :P], junk, start=True, stop=True)

            def v_chain(tt, ne):
                ps = psA.tile(
                    [P, 512], fp32, name=f"pv{tt}_{ne}", tag="proj", bufs=4
                )
                for d in range(N_DT):
                    nc.tensor.matmul(
                        ps,
                        ctxg[d // 4][:, tt, d % 4, :],
                        wvg[d // 4][:, 4 * ne : 4 * ne + 4, d % 4, :],
                        start=(d == 0),
                        stop=(d == N_DT - 1),
                    )
                nc.scalar.copy(out=v[tt][:, ne * 512 : (ne + 1) * 512], in_=ps)

            # ---- Wv^T + ctx^T prep, v = ctx @ Wv^T chases the transposes --
            # Wv rows 0-511 first: v[0][ne=0] needs only those + ctx0, so the
            # first real (HAM-warming) matmul chain starts ~4us earlier and
            # bridges the clock gate while wv4-7 / ctx1-2 stream in.
            for rt in range(4):
                prep_wv(rt)
            prep_ctx(0)
            v_chain(0, 0)
            for rt in range(4, N_DT):
                prep_wv(rt)
            v_chain(0, 1)
            prep_ctx(1)
            prep_ctx(2)

            wstg = {}

            def w_issue(which, tensor, e):
                st_t = stage.tile([P, D], fp32, name=f"ld_{which}{e}", tag="stage")
                nc.sync.dma_start(out=st_t, in_=tensor[e * P : (e + 1) * P, :])
                wstg[(which, e)] = st_t

            def w_cast(which, e):
                bt = wnp.tile([P, D], bf16, name=f"bf_{which}{e}", tag="wn")
                nc.scalar.copy(out=bt, in_=wstg.pop((which, e)))
                return bt

            wqn = [None] * N_DT
            wkn = [None] * N_DT
            for rt in range(1, N_TT):
                # keep ctx prep two tiles ahead of the consuming matmuls
                if rt + 2 < N_TT:
                    prep_ctx(rt + 2)
                # weight loads trickle 2/iter behind the ctx stream; their
                # casts (ACT, interleaved with v-copies) trail by 2 iters so
                # the 3-slot stage rotation never blocks a ctx load
                if 8 <= rt < 12:
                    e = (rt - 8) * 2
                    w_issue("wq", wq, e)
                    w_issue("wq", wq, e + 1)
                elif rt >= 12:
                    e = (rt - 12) * 2
                    w_issue("wk", wk, e)
                    w_issue("wk", wk, e + 1)
                if 10 <= rt < 14:
                    e = (rt - 10) * 2
                    wqn[e] = w_cast("wq", e)
                    wqn[e + 1] = w_cast("wq", e + 1)
                elif rt >= 14:
                    e = (rt - 14) * 2
                    wkn[e] = w_cast("wk", e)
                    wkn[e + 1] = w_cast("wk", e + 1)
                v_chain(rt, 0)
                v_chain(rt, 1)
            for e in range(4, N_DT):
                wkn[e] = w_cast("wk", e)

            def xload(sb):
                return [
                    load_cast(x[(4 * sb + r) * P : (4 * sb + r + 1) * P, :], f"x{4 * sb + r}")
                    for r in range(4)
                ]

            xbs = {0: xload(0)}

            # ---- W' = Wq^T @ Wk from natural layouts ----
            wpb = [
                wpp.tile([P, D], bf16, name=f"wp{i}", tag="wp") for i in range(N_DT)
            ]
            for it in range(N_DT):
                for jn in range(NPROJ):
                    ps = psA.tile(
                        [P, 512], fp32, name=f"pw{it}_{jn}", tag="proj", bufs=4
                    )
                    for e in range(N_DT):
                        nc.tensor.matmul(
                            ps,
                            wqn[e][:, it * P : (it + 1) * P],
                            wkn[e][:, jn * 512 : (jn + 1) * 512],
                            start=(e == 0),
                            stop=(e == N_DT - 1),
                        )
                    nc.scalar.copy(out=wpb[it][:, jn * 512 : (jn + 1) * 512], in_=ps)

            # ---- yT = (x @ W')^T = W'^T x^T, streamed over 4 s-blocks ----
            yt = [
                ytp.tile([P, S], bf16, name=f"yt{j}", tag="yt") for j in range(N_DT)
            ]
            xtb = {}

            def xtrans(sb, xb_list):
                xtb[sb] = [
                    xop.tile([P, 4, 4, P], bf16, name=f"xtb{sb}_{g}", tag="xtb")
                    for g in range(2)
                ]
                for r in range(4):
                    emit_T(xb_list[r], xtb[sb], r, psA, f"x{4 * sb + r}")

            for sb in range(NSB):
                # transposes (and their DVE drains) are emitted before the
                # next block's casts so the drains aren't stuck behind casts
                # in the DVE FIFO when the first yT chain needs them
                xtrans(sb, xbs.pop(sb))
                if sb + 1 < NSB:
                    xbs[sb + 1] = xload(sb + 1)
                for jt in range(N_DT):
                    ps = psA.tile(
                        [P, 512], fp32, name=f"py{sb}_{jt}", tag="proj", bufs=4
                    )
                    for i in range(N_DT):
                        nc.tensor.matmul(
                            ps,
                            wpb[i][:, jt * P : (jt + 1) * P],
                            xtb[sb][i // 4][:, :, i % 4, :],
                            start=(i == 0),
                            stop=(i == N_DT - 1),
                        )
                    nc.scalar.copy(out=yt[jt][:, sb * 512 : (sb + 1) * 512], in_=ps)
                del xtb[sb]

        # ---- attention ----
        # dots is produced TRANSPOSED: dotsT[t_tile, s] = sum_d ctxT[d, t] *
        # yT[d, s] (same operands as dots, roles swapped), so exp output IS
        # attn^T and the attn@v matmul needs no transposes at all. Softmax
        # denominators: a running fp32 elementwise sum of the 16 attn^T
        # tiles on the DVE, then one tiny fp32 ones-matmul per s-tile flips
        # the 128-partition reduction into a [128, 1] column. S is processed
        # in 2 halves of 1024 so attn^T fits the 16 slots Wq/Wk vacated.
        SH = S // 2
        with tc.tile_pool(name="psumB", bufs=1, space="PSUM") as psB:
            for h in range(2):
                atT = []
                spart = spp.tile([P, SH], fp32, name=f"sp{h}", tag="sp")
                for tt in range(N_TT):
                    at = wnp.tile([P, SH], bf16, name=f"atT{h}_{tt}", tag="wn")
                    for ns in range(SH // 512):
                        ps = psB.tile(
                            [P, 512], fp32, name=f"pd{h}_{tt}_{ns}", tag="dots",
                            bufs=4,
                        )
                        for d in range(N_DT):
                            nc.tensor.matmul(
                                ps,
                                ctxg[d // 4][:, tt, d % 4, :],
                                yt[d][:, h * SH + ns * 512 : h * SH + (ns + 1) * 512],
                                start=(d == 0),
                                stop=(d == N_DT - 1),
                            )
                        nc.scalar.activation(
                            out=at[:, ns * 512 : (ns + 1) * 512],
                            in_=ps,
                            func=mybir.ActivationFunctionType.Exp,
                            scale=SCALE,
                        )
                    atT.append(at)
                    # fp32 running column-sum on the DVE chases the exps
                    if tt == 0:
                        nc.vector.tensor_copy(out=spart, in_=at)
                    else:
                        nc.vector.tensor_add(out=spart, in0=spart, in1=at)
                # one bf16 rounding on the denominator (~0.1% rms, budget 2e-2)
                # buys a single-pass LDW for the tiny per-s-tile sum matmuls
                spb = spp.tile([P, SH], bf16, name=f"spb{h}", tag="spb", bufs=1)
                nc.vector.tensor_copy(out=spb, in_=spart)

                def emit_sum(sl):
                    psc = psB.tile(
                        [P, 1], fp32, name=f"psc{h}_{sl}", tag="scol", bufs=1
                    )
                    nc.tensor.matmul(
                        psc,
                        spb[:, sl * P : (sl + 1) * P],
                        ones_b,
                        start=True,
                        stop=True,
                    )
                    recip = smp.tile([P, 1], fp32, name=f"rc{h}_{sl}", tag="recip")
                    nc.vector.reciprocal(out=recip, in_=psc)
                    return recip

                # The tiny denominator matmul for each s-tile rides between the
                # long attn@v chains so the PE never stalls on the DVE running
                # sum; the output DMA is split per 512-column half so the last
                # tile's store overlaps its second normalization.
                recip0 = None
                for sl in range(N_ST // 2):
                    st = h * (N_ST // 2) + sl
                    # sl=0's sum matmul waits for the DVE running sum, so it
                    # rides after sl=0's chains; later ones ride ahead of
                    # their chains so the reciprocal is long done by the mul.
                    if sl > 0:
                        recip = emit_sum(sl)
                    out_sb = xop.tile([P, D], fp32, name=f"o{st}", tag="xtb")
                    pav = []
                    for ne in range(NPROJ):
                        ps = psB.tile(
                            [P, 512], fp32, name=f"pav{st}_{ne}", tag="av", bufs=3
                        )
                        for tt in range(N_TT):
                            nc.tensor.matmul(
                                ps,
                                atT[tt][:, sl * P : (sl + 1) * P],
                                v[tt][:, ne * 512 : (ne + 1) * 512],
                                start=(tt == 0),
                                stop=(tt == N_TT - 1),
                            )
                        pav.append(ps)
                    if sl == 0:
                        recip = emit_sum(0)
                    for ne in range(NPROJ):
                        nc.vector.tensor_scalar_mul(
                            out_sb[:, ne * 512 : (ne + 1) * 512],
                            pav[ne],
                            recip,
                        )
                        nc.sync.dma_start(
                            out=out[st * P : (st + 1) * P, ne * 512 : (ne + 1) * 512],
                            in_=out_sb[:, ne * 512 : (ne + 1) * 512],
                        )


def build_nc():
    import concourse.mybir as mybir
    import concourse.tile as tile
    from concourse import bacc

    fp32 = mybir.dt.float32
    nc = bacc.Bacc("TRN2", target_bir_lowering=False, debug=False)
    x = nc.dram_tensor("x", [S, D], fp32, kind="ExternalInput").ap()
    ctxt = nc.dram_tensor("context", [T, D], fp32, kind="ExternalInput").ap()
    wq = nc.dram_tensor("Wq", [D, D], fp32, kind="ExternalInput").ap()
    wk = nc.dram_tensor("Wk", [D, D], fp32, kind="ExternalInput").ap()
    wv = nc.dram_tensor("Wv", [D, D], fp32, kind="ExternalInput").ap()
    out = nc.dram_tensor("out", [S, D], fp32, kind="ExternalOutput").ap()
    with tile.TileContext(nc) as tc:
        _emit_body(tc, x, ctxt, wq, wk, wv, out)
    nc.compile()
    return nc


_CACHED_NC = None


def kernel(**inputs):
    global _CACHED_NC
    from concourse.bass_utils import run_bass_kernel_spmd

    x = np.ascontiguousarray(np.asarray(inputs["x"], dtype=np.float32))
    ctxt = np.ascontiguousarray(np.asarray(inputs["context"], dtype=np.float32))
    wq = np.ascontiguousarray(np.asarray(inputs["Wq"], dtype=np.float32))
    wk = np.ascontiguousarray(np.asarray(inputs["Wk"], dtype=np.float32))
    wv = np.ascontiguousarray(np.asarray(inputs["Wv"], dtype=np.float32))

    if _CACHED_NC is None:
        _CACHED_NC = build_nc()
    nc = _CACHED_NC

    in_maps = [
        {"x": x[b], "context": ctxt[b], "Wq": wq, "Wk": wk, "Wv": wv}
        for b in range(B)
    ]
    res = run_bass_kernel_spmd(nc, in_maps, core_ids=list(range(B)))
    return np.stack([res.results[b]["out"] for b in range(B)], axis=0)
